# revision 24
# baseline (speedup 1.0000x reference)
"""Trainium2 Bass kernel for nn_CMLITargetLoss (CMLI target loss).

Data-parallel over batch: 64 samples -> 8 NeuronCores x 8 samples.

Host ships, per core:
  textT/targetT/imageT : [768, 8, 197] f32  (D-major transposed local batch)
  keepA0 [128,8], keepA1 [68,8] : keep-mask columns for t=1..128 / 129..196
  keeprow [1, 8*197]            : keep-mask row (b-major, t=0 forced 0)
Device computes per-core partial sums (one [128, 24] f32 tile), host combines
the 8 partial tiles into the final scalar loss (the "all-reduce" of scalars).

Per-core device pipeline (engine assignment tuned against the CoreSim
cost model; DMA ~40us is the roofline):
  r2_n  = sum_d target^2  (ACT Square->fp16 + PE ones-matmul rows; fp16 keeps
          r2 relative error ~1e-5, well under the ~3e-5 min argmax margin)
  rinv  = 1/sqrt(r2)      (PE row->col, DVE reciprocal + ACT sqrt + one
          Newton step, exact reciprocal last)
  rep   = rinv broadcast over partitions (PE outer product via stride-0
          broadcast stationary operand x identity)
  sim   = text[1:] @ target[1:]^T per sample, fp32 (PE, 12 matmuls/sample)
  scaled= sim * rep; m = rowmax(scaled)
  A     = sum_n (scaled>=m)*sim_raw   (fused scalar_tensor_tensor = raw dot
          at the argmax; one-hot comparison is exact)
  B     = (A/m)^2                     (= ||target_argmax||^2)
  S1    = sum keep*(||text_t||^2 - 2A + B)   (text norms via bf16 rows)
  S2    = sum (text[:,0]-target[:,0])^2 ; S3 = sum (image-target)^2
"""

import os
import sys

import numpy as np

for _p in ("/opt/trn_rl_repo", "/root/.axon_site/_ro/trn_rl_repo"):
    if os.path.isdir(_p) and _p not in sys.path:
        sys.path.insert(0, _p)

B, T, D = 64, 197, 768
NC_ = 8            # cores
BL = B // NC_      # 8 local samples per core
KD = D // 128      # 6 d-chunks
TM1 = T - 1        # 196
C0, C1 = 128, TM1 - 128   # t-chunk sizes 128, 68
N0, N1 = 128, T - 128     # n-chunk sizes 128, 69 (n = 0..196)

_CACHE = {}


def _build():
    import concourse.bacc as bacc
    import concourse.tile as tile
    from concourse import mybir
    from concourse.masks import make_identity
    from contextlib import ExitStack

    f32 = mybir.dt.float32
    f16 = mybir.dt.float16
    bf16 = mybir.dt.bfloat16
    Alu = mybir.AluOpType
    Act = mybir.ActivationFunctionType

    nc = bacc.Bacc("TRN2", target_bir_lowering=False, debug=False)

    tT = nc.dram_tensor("textT", (D, BL, T), f32, kind="ExternalInput")
    gT = nc.dram_tensor("targetT", (D, BL, T), f32, kind="ExternalInput")
    iT = nc.dram_tensor("imageT", (D, BL, T), f32, kind="ExternalInput")
    keepA0 = nc.dram_tensor("keepA0", (C0, BL), f32, kind="ExternalInput")
    keepA1 = nc.dram_tensor("keepA1", (C1, BL), f32, kind="ExternalInput")
    keeprow = nc.dram_tensor("keeprow", (1, BL * T), f32, kind="ExternalInput")
    outp = nc.dram_tensor("partials", (128, 24), f32, kind="ExternalOutput")

    NROW = 2 * T  # 394: two samples per psum row region

    with tile.TileContext(nc) as tc, ExitStack() as ctx:
        consts = ctx.enter_context(tc.tile_pool(name="consts", bufs=1))
        inputs = ctx.enter_context(tc.tile_pool(name="inputs", bufs=1))
        sbuf = ctx.enter_context(tc.tile_pool(name="sbuf", bufs=1))
        scratch = ctx.enter_context(tc.tile_pool(name="scratch", bufs=2))
        # one PSUM pool: tag "rowps" = 4 x 1-bank slots (row reductions, small
        # transposes, replicate); tag "simps" = 2 x 2-bank slots (sim psum).
        psum = ctx.enter_context(tc.tile_pool(name="psum", bufs=1, space="PSUM"))

        ident = consts.tile([128, 128], f32, tag="ident")
        make_identity(nc, ident)
        ones_row = consts.tile([1, 128], f32, tag="ones_row")
        nc.vector.memset(ones_row, 1.0)
        ones_col_bf = consts.tile([128, 1], bf16, tag="ones_col_bf")
        nc.vector.memset(ones_col_bf, 1.0)
        ones_col_f16 = consts.tile([128, 1], f16, tag="ones_col_f16")
        nc.vector.memset(ones_col_f16, 1.0)

        kA0 = consts.tile([C0, BL], f32, tag="kA0")
        nc.sync.dma_start(out=kA0, in_=keepA0[:, :])
        kA1 = consts.tile([C1, BL], f32, tag="kA1")
        nc.sync.dma_start(out=kA1, in_=keepA1[:, :])
        krow = consts.tile([1, BL * T], f32, tag="krow")
        nc.sync.dma_start(out=krow, in_=keeprow[:, :])

        ost = consts.tile([128, 24], f32, tag="ost")
        nc.vector.memset(ost, 0.0)

        # ACT table preload (sqrt set also contains Square) + PE warmup to
        # ramp the clock gate before the first real matmul.
        actwarm = consts.tile([1, 1], f32, tag="actwarm")
        nc.vector.memset(actwarm, 1.0)
        actwarm2 = consts.tile([1, 1], f32, tag="actwarm2")
        nc.scalar.activation(actwarm2, actwarm, Act.Sqrt)
        warm_ps = psum.tile([128, 128], f32, tag="rowps0", name="warm_ps")
        for w in range(30):
            nc.tensor.matmul(warm_ps, ident, ident, start=True, stop=True,
                             skip_group_check=True)

        # resident inputs: [128, KD, BL, T] (chunk k on free axis 0)
        t_all = inputs.tile([128, KD, BL, T], f32, tag="t_all")
        g_all = inputs.tile([128, KD, BL, T], f32, tag="g_all")
        i_all = inputs.tile([128, KD, BL, T], f32, tag="i_all")

        def _ld(dst, srcdram, k):
            nc.sync.dma_start(
                out=dst[:, k].rearrange("p b t -> p (b t)"),
                in_=srcdram[k * 128:(k + 1) * 128].rearrange("p b t -> p (b t)"))
        for k in range(KD):
            _ld(g_all, gT, k)
        for k in range(KD):
            _ld(t_all, tT, k)
        for k in range(KD):
            _ld(i_all, iT, k)

        # ---------------- r2 rows: r2[b, n] = sum_d target^2 (fp16) ----------
        r2row_ps = [psum.tile([1, NROW], f32, tag=f"rowps{j}", name=f"r2row_ps{j}")
                    for j in range(4)]
        for k in range(KD):
            sq = scratch.tile([128, BL, T], f16, tag="sqbf")
            nc.scalar.activation(sq, g_all[:, k], Act.Square)
            for j in range(4):
                nc.tensor.matmul(
                    r2row_ps[j], ones_col_f16, sq[:, 2 * j:2 * j + 2, :],
                    start=(k == 0), stop=(k == KD - 1),
                )
        # textsq ACT squares early (t-chunk cadence); PE row-matmuls run mid-sim
        tsq_tiles = []
        for k in range(KD):
            tsq_k = scratch.tile([128, BL, T], bf16, tag="tsqq", bufs=KD,
                                 name=f"tsq_{k}")
            nc.scalar.activation(tsq_k, t_all[:, k], Act.Square)
            tsq_tiles.append(tsq_k)

        r2row = sbuf.tile([1, BL * T], f32, tag="r2row")
        for j in range(4):
            nc.vector.tensor_copy(r2row[:, j * NROW:(j + 1) * NROW], r2row_ps[j])

        # rows -> columns (per sample)
        r2c0_ps = psum.tile([128, BL], f32, tag="rowps0")
        for b in range(BL):
            nc.tensor.matmul(
                r2c0_ps[:, b:b + 1], r2row[:, b * T:b * T + N0], ones_row[:, 0:1],
                skip_group_check=True,
            )
        r2c0 = sbuf.tile([128, BL], f32, tag="r2c0")
        nc.vector.tensor_copy(r2c0, r2c0_ps)
        r2c1_ps = psum.tile([N1, BL], f32, tag="rowps1")
        for b in range(BL):
            nc.tensor.matmul(
                r2c1_ps[:, b:b + 1], r2row[:, b * T + N0:(b + 1) * T], ones_row[:, 0:1],
                skip_group_check=True,
            )
        r2c1 = sbuf.tile([N1, BL], f32, tag="r2c1")
        nc.vector.tensor_copy(r2c1, r2c1_ps)

        # rinv = 1/sqrt(r2): ACT sqrt seed + one Newton step + exact reciprocal
        def _rinv(r2c, P, tag):
            r0 = sbuf.tile([P, BL], f32, tag=tag + "_r0")
            nc.scalar.activation(r0, r2c, Act.Sqrt)
            ir = scratch.tile([P, BL], f32, tag=tag + "_t", bufs=4)
            nc.vector.reciprocal(ir, r0)
            t = scratch.tile([P, BL], f32, tag=tag + "_t", bufs=4)
            nc.vector.tensor_mul(t, r2c, ir)                   # r2/r0
            a = scratch.tile([P, BL], f32, tag=tag + "_t", bufs=4)
            nc.vector.tensor_add(a, r0, t)
            r1 = sbuf.tile([P, BL], f32, tag=tag + "_r1")
            nc.vector.tensor_scalar(out=r1, in0=a, scalar1=0.5, scalar2=None,
                                    op0=Alu.mult)              # 0.5*(r0 + r2/r0)
            rinv = sbuf.tile([P, BL], f32, tag=tag)
            nc.vector.reciprocal(rinv, r1)
            return rinv

        rinv0 = _rinv(r2c0, 128, "rinv0")
        rinv1 = _rinv(r2c1, N1, "rinv1")

        # ---------------- cls token loss -> out c17..c22 ----------------
        for k in range(KD):
            d0 = scratch.tile([128, BL], f32, tag="cls")
            nc.vector.tensor_sub(d0, t_all[:, k, :, 0], g_all[:, k, :, 0])
            d0sq = scratch.tile([128, BL], f32, tag="cls")
            nc.scalar.activation(d0sq, d0, Act.Square,
                                 accum_out=ost[:, 17 + k:18 + k])

        # ---------------- similarity + argmax extraction ----------------
        Mc0 = sbuf.tile([128, BL], f32, tag="Mc0")
        Mc1 = sbuf.tile([C1, BL], f32, tag="Mc1")
        Ac0 = sbuf.tile([128, BL], f32, tag="Ac0")
        Ac1 = sbuf.tile([C1, BL], f32, tag="Ac1")

        sim_ps = {}

        def _sim_mms(grp):
            b0 = grp * GS
            for ci, (P, lo, hi) in enumerate(((128, 1, 1 + C0), (C1, 1 + C0, T))):
                ps = psum.tile([128, GS, 256], f32, tag="simps",
                               name=f"ps_{grp}_{ci}", bufs=4)
                sim_ps[(grp, ci)] = ps
                for g in range(GS):
                    b = b0 + g
                    for k in range(KD):
                        nc.tensor.matmul(
                            ps[:P, g, 0:TM1],
                            t_all[:, k, b, lo:hi], g_all[:, k, b, 1:T],
                            start=(k == 0), stop=(k == KD - 1),
                        )

        def _sim_post(grp):
            b0 = grp * GS
            for ci, (P, Mc, Ac) in enumerate(((128, Mc0, Ac0), (C1, Mc1, Ac1))):
                ps = sim_ps[(grp, ci)]
                ss = scratch.tile([128, GS, TM1], f32, tag="ss", bufs=3)
                nc.vector.tensor_mul(ss[:P], ps[:P, :, 0:TM1],
                                     rep[:P, b0:b0 + GS, :])
                nc.vector.reduce_max(Mc[:, b0:b0 + GS], ss[:P],
                                     axis=mybir.AxisListType.X)
                for g in range(GS):
                    b = b0 + g
                    scr = scratch.tile([128, TM1], f32, tag="scr", bufs=2)
                    nc.vector.scalar_tensor_tensor(
                        out=scr[:P], in0=ss[:P, g, :], scalar=Mc[:, b:b + 1],
                        in1=ps[:P, g, 0:TM1],
                        op0=Alu.is_ge, op1=Alu.mult,
                        accum_out=Ac[:, b:b + 1],
                    )

        GS = 2  # samples per psum group
        _sim_mms(0)
        _sim_mms(1)

        # replicate rinv (n = 1..196) across partitions: rep[p, b, j] = rinv[b, j+1]
        # (emitted here so the PE does it during sim's t-chunk stalls)
        rep = sbuf.tile([128, BL, TM1], f32, tag="rep")
        for b in range(BL):
            rep_ps0 = psum.tile([128, N0], f32, tag=f"rowps{(2 * b) % 4}",
                                name=f"rep_ps0_{b}")
            nc.tensor.matmul(rep_ps0, rinv0[:, b:b + 1].broadcast_to([N0, 128]),
                             ident)
            nc.vector.tensor_copy(rep[:, b, 0:127], rep_ps0[:, 1:128])
            rep_ps1 = psum.tile([128, N1], f32, tag=f"rowps{(2 * b + 1) % 4}",
                                name=f"rep_ps1_{b}")
            nc.tensor.matmul(rep_ps1, rinv1[:, b:b + 1].broadcast_to([N1, 128]),
                             ident[:N1, :N1])
            nc.vector.tensor_copy(rep[:, b, 127:TM1], rep_ps1[:, 0:N1])

        _sim_post(0)
        _sim_post(1)

        # textsq PE row reduction (squares precomputed; all t resident by now)
        tsqrow_ps = [psum.tile([1, NROW], f32, tag=f"rowps{j}", name=f"tsqrow_ps{j}")
                     for j in range(4)]
        for k in range(KD):
            for j in range(4):
                nc.tensor.matmul(
                    tsqrow_ps[j], ones_col_bf, tsq_tiles[k][:, 2 * j:2 * j + 2, :],
                    start=(k == 0), stop=(k == KD - 1),
                )
        tsqrow = sbuf.tile([1, BL * T], f32, tag="tsqrow")
        for j in range(4):
            nc.vector.tensor_copy(tsqrow[:, j * NROW:(j + 1) * NROW], tsqrow_ps[j])
        trsc = sbuf.tile([1, BL * T], f32, tag="trsc")
        nc.vector.scalar_tensor_tensor(
            out=trsc, in0=tsqrow, scalar=1.0, in1=krow,
            op0=Alu.mult, op1=Alu.mult, accum_out=ost[0:1, 2:3],
        )

        for grp in range(2, BL // GS):
            _sim_mms(grp)
            _sim_post(grp)

        # column math: S1 partial = sum keep * (B - 2A), B = (A/m)^2
        for P, Mc, Ac, kA, col in ((128, Mc0, Ac0, kA0, 0), (C1, Mc1, Ac1, kA1, 1)):
            rm = scratch.tile([128, BL], f32, tag="colm", bufs=8)
            nc.vector.reciprocal(rm[:P], Mc)                       # 1/m
            q = scratch.tile([128, BL], f32, tag="colm", bufs=8)
            nc.vector.tensor_mul(q[:P], Ac, rm[:P])                # A/m = r_sel
            bb = scratch.tile([128, BL], f32, tag="colm", bufs=8)
            nc.vector.tensor_mul(bb[:P], q[:P], q[:P])             # r_sel^2
            a2 = scratch.tile([128, BL], f32, tag="colm", bufs=8)
            nc.vector.tensor_scalar(out=a2[:P], in0=Ac, scalar1=-2.0, scalar2=None,
                                    op0=Alu.mult)
            v2 = scratch.tile([128, BL], f32, tag="colm", bufs=8)
            nc.vector.tensor_add(v2[:P], bb[:P], a2[:P])
            v2k = scratch.tile([128, BL], f32, tag="colm", bufs=8)
            nc.vector.scalar_tensor_tensor(
                out=v2k[:P], in0=v2[:P], scalar=1.0, in1=kA,
                op0=Alu.mult, op1=Alu.mult, accum_out=ost[:P, col:col + 1],
            )

        # ---------------- image loss: sum (image-target)^2 -> c3..8 ----------
        for k in range(KD):
            df = scratch.tile([128, BL, T], f32, tag="sq")
            eng = nc.vector if k % 2 == 0 else nc.gpsimd
            eng.tensor_sub(df, i_all[:, k], g_all[:, k])
            dfsq = scratch.tile([128, BL, T], f32, tag="sq")
            nc.scalar.activation(dfsq, df, Act.Square,
                                 accum_out=ost[:, 3 + k:4 + k])

        # ---------------- text row norms (bf16), masked sum -> c2 ------------
        nc.sync.dma_start(out=outp[:, :], in_=ost)

    nc.compile()
    return nc


def _get_nc():
    if "nc" not in _CACHE:
        _CACHE["nc"] = _build()
    return _CACHE["nc"]


def _prepare(image, text, target, padding_mask):
    image = np.ascontiguousarray(np.asarray(image, dtype=np.float32))
    text = np.ascontiguousarray(np.asarray(text, dtype=np.float32))
    target = np.ascontiguousarray(np.asarray(target, dtype=np.float32))
    mask = np.asarray(padding_mask)

    keep = (mask[:, 1:] == 0)          # [B, 196] bool
    n_tokens = float(keep.sum())

    in_maps = []
    for c in range(NC_):
        sl = slice(c * BL, (c + 1) * BL)
        kb = keep[sl].astype(np.float32)            # [BL, 196]
        krow = np.zeros((BL, T), np.float32)
        krow[:, 1:] = kb
        in_maps.append({
            "textT": np.ascontiguousarray(text[sl].transpose(2, 0, 1)),
            "targetT": np.ascontiguousarray(target[sl].transpose(2, 0, 1)),
            "imageT": np.ascontiguousarray(image[sl].transpose(2, 0, 1)),
            "keepA0": np.ascontiguousarray(kb[:, 0:C0].T),
            "keepA1": np.ascontiguousarray(kb[:, C0:TM1].T),
            "keeprow": krow.reshape(1, BL * T),
        })
    return in_maps, n_tokens


def _combine(results, n_tokens):
    S1 = S2 = S3 = 0.0
    for r in results:
        P = r["partials"].astype(np.float64)
        S1 += P[:, 0].sum() + P[:, 1].sum() + P[0, 2]
        S3 += P[:, 3:9].sum()
        S2 += P[:, 17:23].sum()

    kd_tok = S1 / (n_tokens * D)
    kd_cls = S2 / (B * D)
    kd_text = (n_tokens * kd_tok + kd_cls) / (n_tokens + 1.0)
    kd_img = S3 / (B * T * D)
    return np.float32((kd_text + kd_img) / 2.0)


def kernel(image, text, target, padding_mask):
    from concourse.bass_utils import run_bass_kernel_spmd

    in_maps, n_tokens = _prepare(image, text, target, padding_mask)
    nc = _get_nc()
    results = run_bass_kernel_spmd(nc, in_maps, core_ids=list(range(NC_))).results
    return _combine(results, n_tokens)
